# revision 5
# baseline (speedup 1.0000x reference)
"""Trainium2 Bass kernel for HDGradientCompressionLayer forward.

Reference computation: y = einsum("bsd,df->bsf", x, W) + b
  x: (4, 4096, 1024) f32, W: (1024, 1024) f32, b: (1024,) f32.

Strategy (data-parallel across 8 cores, per sharding hint):
  Flatten x to (16384, 1024); each core gets 2048 rows. Per core the
  kernel computes y_shard = x_shard @ W + b:
    - x rowblock [128, 1024] is cast-loaded f32->bf16 (SWDGE cast DMA),
    - one xbar DMA-transpose produces the 8 stationary [d,row] tiles,
    - 16 bf16 matmuls (N=512, PSUM-accumulated over the 8 d-blocks),
    - DVE adds the (partition-broadcast) f32 bias during PSUM->SBUF
      eviction, stores f32 y rowblock.
  W is cast-loaded once, d-blocked to match the transposed x layout.
"""

import os
from contextlib import ExitStack

import numpy as np

import concourse.bass as bass
import concourse.bacc as bacc
import concourse.tile as tile
from concourse import mybir
from concourse.bass_utils import run_bass_kernel_spmd

N_CORES = 8
B, S, D = 4, 4096, 1024
F = 1024
ROWS_TOTAL = B * S          # 16384
ROWS = ROWS_TOTAL // N_CORES  # 2048 per core
P = 128
NSPLIT = 512                # one PSUM bank of f32


def build_nc(rows: int = ROWS) -> bass.Bass:
    nc = bacc.Bacc("TRN2", target_bir_lowering=False, debug=False)
    x = nc.dram_tensor("x", [rows, D], mybir.dt.float32, kind="ExternalInput").ap()
    W = nc.dram_tensor("W", [D, F], mybir.dt.float32, kind="ExternalInput").ap()
    b = nc.dram_tensor("b", [F], mybir.dt.float32, kind="ExternalInput").ap()
    y = nc.dram_tensor("y", [rows, F], mybir.dt.float32, kind="ExternalOutput").ap()

    KB = D // P        # 8 contraction blocks
    NB = F // NSPLIT   # 2 psum banks per rowblock
    RB = rows // P     # rowblocks

    with tile.TileContext(nc) as tc, ExitStack() as ctx:
        const = ctx.enter_context(tc.tile_pool(name="const", bufs=1))
        xp = ctx.enter_context(tc.tile_pool(name="xp", bufs=4))
        xtp = ctx.enter_context(tc.tile_pool(name="xtp", bufs=4))
        yp = ctx.enter_context(tc.tile_pool(name="yp", bufs=4))
        psp = ctx.enter_context(tc.tile_pool(name="psp", bufs=8, space="PSUM"))

        # W, cast to bf16, laid out [p, k, f] with d = k*128 + p to match
        # the xbar-transpose output layout of x.
        W_bf = const.tile([P, KB, F], mybir.dt.bfloat16)
        nc.gpsimd.dma_start(W_bf[:], W.rearrange("(k p) f -> p k f", p=P))

        # Bias broadcast to all partitions, f32.
        b_bc = const.tile([P, F], mybir.dt.float32)
        nc.gpsimd.dma_start(b_bc[:], b.rearrange("(o f) -> o f", o=1).to_broadcast([P, F]))

        for rb in range(RB):
            x_bf = xp.tile([P, D], mybir.dt.bfloat16)
            nc.gpsimd.dma_start(x_bf[:], x[rb * P:(rb + 1) * P, :])  # cast load
            xT = xtp.tile([P, KB, P], mybir.dt.bfloat16)
            # xT[p, k, j] = x_bf[j, k*128+p]
            nc.sync.dma_start(xT[:], x_bf[:], transpose=True)

            y_sb = yp.tile([P, F], mybir.dt.float32)
            for n in range(NB):
                ps = psp.tile([P, NSPLIT], mybir.dt.float32)
                for k in range(KB):
                    nc.tensor.matmul(
                        ps[:],
                        xT[:, k, :],
                        W_bf[:, k, n * NSPLIT:(n + 1) * NSPLIT],
                        start=(k == 0),
                        stop=(k == KB - 1),
                    )
                nc.vector.tensor_add(
                    y_sb[:, n * NSPLIT:(n + 1) * NSPLIT],
                    ps[:],
                    b_bc[:, n * NSPLIT:(n + 1) * NSPLIT],
                )
            nc.sync.dma_start(y[rb * P:(rb + 1) * P, :], y_sb[:])

    nc.compile()
    return nc


_NC_CACHE: dict[int, bass.Bass] = {}


def _get_nc(rows: int = ROWS) -> bass.Bass:
    if rows not in _NC_CACHE:
        _NC_CACHE[rows] = build_nc(rows)
    return _NC_CACHE[rows]


def _run(in_maps, rows: int = ROWS, trace: bool = False):
    nc = _get_nc(rows)
    return run_bass_kernel_spmd(nc, in_maps, list(range(N_CORES)), trace=trace)


def kernel(x: np.ndarray, W: np.ndarray, b: np.ndarray) -> np.ndarray:
    x = np.ascontiguousarray(np.asarray(x, dtype=np.float32))
    W = np.ascontiguousarray(np.asarray(W, dtype=np.float32))
    b = np.ascontiguousarray(np.asarray(b, dtype=np.float32))
    x_flat = x.reshape(ROWS_TOTAL, D)
    in_maps = [
        {"x": np.ascontiguousarray(x_flat[c * ROWS:(c + 1) * ROWS]), "W": W, "b": b}
        for c in range(N_CORES)
    ]
    res = _run(in_maps, trace=bool(int(os.environ.get("BASS_KERNEL_TRACE", "0"))))
    y = np.concatenate([res.results[c]["y"] for c in range(N_CORES)], axis=0)
    return y.reshape(B, S, F)


# revision 9
# speedup vs baseline: 1.1202x; 1.1202x over previous
"""Trainium2 Bass kernel for HDGradientCompressionLayer forward.

Reference computation: y = einsum("bsd,df->bsf", x, W) + b
  x: (4, 4096, 1024) f32, W: (1024, 1024) f32, b: (1024,) f32.

Strategy (data-parallel across 8 cores, per sharding hint):
  Flatten x to (16384, 1024); each core gets 2048 rows. Per core the
  kernel computes y_shard = x_shard @ W + b:
    - x rowblock [128, 1024] is cast-loaded f32->bf16 (SWDGE cast DMA),
    - one xbar DMA-transpose produces the 8 stationary [d,row] tiles,
    - 16 bf16 matmuls (N=512, PSUM-accumulated over the 8 d-blocks),
    - DVE adds the (partition-broadcast) f32 bias during PSUM->SBUF
      eviction, stores f32 y rowblock.
  W is cast-loaded once, d-blocked to match the transposed x layout.
"""

import os
from contextlib import ExitStack

import numpy as np

import concourse.bass as bass
import concourse.bacc as bacc
import concourse.tile as tile
from concourse import mybir
from concourse.bass_utils import run_bass_kernel_spmd

N_CORES = 8
B, S, D = 4, 4096, 1024
F = 1024
ROWS_TOTAL = B * S          # 16384
ROWS = ROWS_TOTAL // N_CORES  # 2048 per core
P = 128
NSPLIT = 512                # one PSUM bank of f32


def build_nc(rows: int = ROWS) -> bass.Bass:
    nc = bacc.Bacc("TRN2", target_bir_lowering=False, debug=False)
    x = nc.dram_tensor("x", [rows, D], mybir.dt.float32, kind="ExternalInput").ap()
    W = nc.dram_tensor("W", [D, F], mybir.dt.float32, kind="ExternalInput").ap()
    b = nc.dram_tensor("b", [F], mybir.dt.float32, kind="ExternalInput").ap()
    y = nc.dram_tensor("y", [rows, F], mybir.dt.float32, kind="ExternalOutput").ap()

    KB = D // P        # 8 contraction blocks
    NB = F // NSPLIT   # 2 psum banks per rowblock
    RB = rows // P     # rowblocks

    with tile.TileContext(nc) as tc, ExitStack() as ctx:
        const = ctx.enter_context(tc.tile_pool(name="const", bufs=1))
        xp = ctx.enter_context(tc.tile_pool(name="xp", bufs=RB))
        xtp = ctx.enter_context(tc.tile_pool(name="xtp", bufs=RB))
        yp = ctx.enter_context(tc.tile_pool(name="yp", bufs=4))
        psp = ctx.enter_context(tc.tile_pool(name="psp", bufs=1, space="PSUM"))

        # W, cast to bf16, laid out [p, k, f] with d = k*128 + p to match
        # the xbar-transpose output layout of x.  Split per k-block so the
        # first matmuls' weights land early.
        W_bf = const.tile([P, KB, F], mybir.dt.bfloat16)
        W_pkf = W.rearrange("(k p) f -> p k f", p=P)

        # Bias broadcast to all partitions, f32.
        b_bc = const.tile([P, F], mybir.dt.float32)

        # HAM warmup: ~10 cold matmuls (~4.3us of PE activity) on a zeroed
        # tile flip the PE clock gate to 8/8 while the first DMAs land.
        warm = const.tile([P, P], mybir.dt.bfloat16)
        nc.any.memset(warm[:], 0.0)
        warm_ps = psp.tile([P, NSPLIT], mybir.dt.float32, tag="warm", bufs=1)
        for _ in range(10):
            nc.tensor.matmul(warm_ps[:], warm[:], warm[:, 0:1].to_broadcast([P, NSPLIT]),
                             start=True, stop=True, skip_group_check=True)

        # Emission order = rough issue order per queue: get k0 of W, the
        # bias, and the first x rowblocks in flight first.
        nc.gpsimd.dma_start(W_bf[:, 0, :], W_pkf[:, 0, :])
        nc.gpsimd.dma_start(b_bc[:], b.rearrange("(o f) -> o f", o=1).to_broadcast([P, F]))

        x_tiles = []
        xt_tiles = []

        def load_rb(rb):
            x_bf = xp.tile([P, D], mybir.dt.bfloat16)
            nc.gpsimd.dma_start(x_bf[:], x[rb * P:(rb + 1) * P, :])  # cast load
            xT = xtp.tile([P, KB, P], mybir.dt.bfloat16)
            # xT[p, k, j] = x_bf[j, k*128+p]
            nc.sync.dma_start(xT[:], x_bf[:], transpose=True)
            x_tiles.append(x_bf)
            xt_tiles.append(xT)

        load_rb(0)
        load_rb(1)
        for k in range(1, KB):
            nc.gpsimd.dma_start(W_bf[:, k, :], W_pkf[:, k, :])
        for rb in range(2, RB):
            load_rb(rb)

        for rb in range(RB):
            xT = xt_tiles[rb]
            y_sb = yp.tile([P, F], mybir.dt.float32)
            pss = [psp.tile([P, NSPLIT], mybir.dt.float32, name=f"ps{n}", tag=f"ps{n}", bufs=3) for n in range(NB)]
            for k in range(KB):
                for n in range(NB):
                    nc.tensor.matmul(
                        pss[n][:],
                        xT[:, k, :],
                        W_bf[:, k, n * NSPLIT:(n + 1) * NSPLIT],
                        start=(k == 0),
                        stop=(k == KB - 1),
                    )
            for n in range(NB):
                nc.vector.tensor_add(
                    y_sb[:, n * NSPLIT:(n + 1) * NSPLIT],
                    pss[n][:],
                    b_bc[:, n * NSPLIT:(n + 1) * NSPLIT],
                )
            nc.scalar.dma_start(y[rb * P:(rb + 1) * P, :], y_sb[:])

    nc.compile()
    return nc


_NC_CACHE: dict[int, bass.Bass] = {}


def _get_nc(rows: int = ROWS) -> bass.Bass:
    if rows not in _NC_CACHE:
        _NC_CACHE[rows] = build_nc(rows)
    return _NC_CACHE[rows]


def _run(in_maps, rows: int = ROWS, trace: bool = False):
    nc = _get_nc(rows)
    return run_bass_kernel_spmd(nc, in_maps, list(range(N_CORES)), trace=trace)


def kernel(x: np.ndarray, W: np.ndarray, b: np.ndarray) -> np.ndarray:
    x = np.ascontiguousarray(np.asarray(x, dtype=np.float32))
    W = np.ascontiguousarray(np.asarray(W, dtype=np.float32))
    b = np.ascontiguousarray(np.asarray(b, dtype=np.float32))
    x_flat = x.reshape(ROWS_TOTAL, D)
    in_maps = [
        {"x": np.ascontiguousarray(x_flat[c * ROWS:(c + 1) * ROWS]), "W": W, "b": b}
        for c in range(N_CORES)
    ]
    res = _run(in_maps, trace=bool(int(os.environ.get("BASS_KERNEL_TRACE", "0"))))
    y = np.concatenate([res.results[c]["y"] for c in range(N_CORES)], axis=0)
    return y.reshape(B, S, F)


# revision 11
# speedup vs baseline: 1.1992x; 1.0705x over previous
"""Trainium2 Bass kernel for HDGradientCompressionLayer forward.

Reference computation: y = einsum("bsd,df->bsf", x, W) + b
  x: (4, 4096, 1024) f32, W: (1024, 1024) f32, b: (1024,) f32.

Strategy (data-parallel across 8 cores, per sharding hint):
  Flatten x to (16384, 1024); each core gets 2048 rows. Per core the
  kernel computes y_shard = x_shard @ W + b:
    - x rowblock [128, 1024] is cast-loaded f32->bf16 (SWDGE cast DMA),
    - one xbar DMA-transpose produces the 8 stationary [d,row] tiles,
    - 16 bf16 matmuls (N=512, PSUM-accumulated over the 8 d-blocks),
    - DVE adds the (partition-broadcast) f32 bias during PSUM->SBUF
      eviction, stores f32 y rowblock.
  W is cast-loaded once, d-blocked to match the transposed x layout.
"""

import os
from contextlib import ExitStack

import numpy as np

import concourse.bass as bass
import concourse.bacc as bacc
import concourse.tile as tile
from concourse import mybir
from concourse.bass_utils import run_bass_kernel_spmd

N_CORES = 8
B, S, D = 4, 4096, 1024
F = 1024
ROWS_TOTAL = B * S          # 16384
ROWS = ROWS_TOTAL // N_CORES  # 2048 per core
P = 128
NSPLIT = 512                # one PSUM bank of f32


def build_nc(rows: int = ROWS) -> bass.Bass:
    nc = bacc.Bacc("TRN2", target_bir_lowering=False, debug=False)
    x = nc.dram_tensor("x", [rows, D], mybir.dt.float32, kind="ExternalInput").ap()
    W = nc.dram_tensor("W", [D, F], mybir.dt.float32, kind="ExternalInput").ap()
    b = nc.dram_tensor("b", [F], mybir.dt.float32, kind="ExternalInput").ap()
    y = nc.dram_tensor("y", [rows, F], mybir.dt.float32, kind="ExternalOutput").ap()

    KB = D // P        # 8 contraction blocks
    NB = F // NSPLIT   # 2 psum banks per rowblock
    RB = rows // P     # rowblocks

    with tile.TileContext(nc) as tc, ExitStack() as ctx:
        const = ctx.enter_context(tc.tile_pool(name="const", bufs=1))
        xp = ctx.enter_context(tc.tile_pool(name="xp", bufs=RB))
        xtp = ctx.enter_context(tc.tile_pool(name="xtp", bufs=RB))
        yp = ctx.enter_context(tc.tile_pool(name="yp", bufs=RB))
        psp = ctx.enter_context(tc.tile_pool(name="psp", bufs=1, space="PSUM"))

        # W, cast to bf16, laid out [p, k, f] with d = k*128 + p to match
        # the xbar-transpose output layout of x.  Split per k-block so the
        # first matmuls' weights land early.
        W_bf = const.tile([P, KB, F], mybir.dt.bfloat16)
        W_pkf = W.rearrange("(k p) f -> p k f", p=P)

        # Bias broadcast to all partitions, f32.
        b_bc = const.tile([P, F], mybir.dt.float32)

        # HAM warmup: ~10 cold matmuls (~4.3us of PE activity) on a zeroed
        # tile flip the PE clock gate to 8/8 while the first DMAs land.
        warm = const.tile([P, P], mybir.dt.bfloat16)
        nc.any.memset(warm[:], 0.0)
        warm_ps = psp.tile([P, NSPLIT], mybir.dt.float32, tag="warm", bufs=1)
        for _ in range(10):
            nc.tensor.matmul(warm_ps[:], warm[:], warm[:, 0:1].to_broadcast([P, NSPLIT]),
                             start=True, stop=True, skip_group_check=True)

        # DMA_TRANSPOSE fences other DMA traffic around it (xbar mode), so
        # group the ops into copy-phases and transpose-phases:
        #   copies [W, b, x0..x3] -> T0..T3 -> copies [x4..] -> T4.. -> stores.
        GROUP1 = min(4, RB)
        x_tiles = []
        xt_tiles = []

        def load_x(rb):
            x_bf = xp.tile([P, D], mybir.dt.bfloat16)
            nc.gpsimd.dma_start(x_bf[:], x[rb * P:(rb + 1) * P, :])  # cast load
            x_tiles.append(x_bf)

        def transpose_x(rb):
            xT = xtp.tile([P, KB, P], mybir.dt.bfloat16)
            # xT[p, k, j] = x_bf[j, k*128+p]
            nc.sync.dma_start(xT[:], x_tiles[rb][:], transpose=True)
            xt_tiles.append(xT)

        nc.gpsimd.dma_start(W_bf[:, 0, :], W_pkf[:, 0, :])
        for rb in range(GROUP1):
            load_x(rb)
        nc.gpsimd.dma_start(b_bc[:], b.rearrange("(o f) -> o f", o=1).to_broadcast([P, F]))
        for k in range(1, KB):
            nc.gpsimd.dma_start(W_bf[:, k, :], W_pkf[:, k, :])
        for rb in range(GROUP1):
            transpose_x(rb)
        for rb in range(GROUP1, RB):
            load_x(rb)
        for rb in range(GROUP1, RB):
            transpose_x(rb)

        for rb in range(RB):
            xT = xt_tiles[rb]
            y_sb = yp.tile([P, F], mybir.dt.float32)
            pss = [psp.tile([P, NSPLIT], mybir.dt.float32, name=f"ps{n}", tag=f"ps{n}", bufs=3) for n in range(NB)]
            for k in range(KB):
                for n in range(NB):
                    nc.tensor.matmul(
                        pss[n][:],
                        xT[:, k, :],
                        W_bf[:, k, n * NSPLIT:(n + 1) * NSPLIT],
                        start=(k == 0),
                        stop=(k == KB - 1),
                    )
            for n in range(NB):
                nc.vector.tensor_add(
                    y_sb[:, n * NSPLIT:(n + 1) * NSPLIT],
                    pss[n][:],
                    b_bc[:, n * NSPLIT:(n + 1) * NSPLIT],
                )
            nc.scalar.dma_start(y[rb * P:(rb + 1) * P, :], y_sb[:])

    nc.compile()
    return nc


_NC_CACHE: dict[int, bass.Bass] = {}


def _get_nc(rows: int = ROWS) -> bass.Bass:
    if rows not in _NC_CACHE:
        _NC_CACHE[rows] = build_nc(rows)
    return _NC_CACHE[rows]


def _run(in_maps, rows: int = ROWS, trace: bool = False):
    nc = _get_nc(rows)
    return run_bass_kernel_spmd(nc, in_maps, list(range(N_CORES)), trace=trace)


def kernel(x: np.ndarray, W: np.ndarray, b: np.ndarray) -> np.ndarray:
    x = np.ascontiguousarray(np.asarray(x, dtype=np.float32))
    W = np.ascontiguousarray(np.asarray(W, dtype=np.float32))
    b = np.ascontiguousarray(np.asarray(b, dtype=np.float32))
    x_flat = x.reshape(ROWS_TOTAL, D)
    in_maps = [
        {"x": np.ascontiguousarray(x_flat[c * ROWS:(c + 1) * ROWS]), "W": W, "b": b}
        for c in range(N_CORES)
    ]
    res = _run(in_maps, trace=bool(int(os.environ.get("BASS_KERNEL_TRACE", "0"))))
    y = np.concatenate([res.results[c]["y"] for c in range(N_CORES)], axis=0)
    return y.reshape(B, S, F)


# revision 13
# speedup vs baseline: 1.6089x; 1.3416x over previous
"""Trainium2 Bass kernel for HDGradientCompressionLayer forward.

Reference computation: y = einsum("bsd,df->bsf", x, W) + b
  x: (4, 4096, 1024) f32, W: (1024, 1024) f32, b: (1024,) f32.

Strategy (data-parallel across 8 cores, per sharding hint):
  Flatten x to (16384, 1024); each core gets 2048 rows. Per core the
  kernel computes y_shard = x_shard @ W + b:
    - x rowblock [128, 1024] is cast-loaded f32->bf16 (SWDGE cast DMA),
    - one xbar DMA-transpose produces the 8 stationary [d,row] tiles,
    - 16 bf16 matmuls (N=512, PSUM-accumulated over the 8 d-blocks),
    - DVE adds the (partition-broadcast) f32 bias during PSUM->SBUF
      eviction, stores f32 y rowblock.
  W is cast-loaded once, d-blocked to match the transposed x layout.
"""

import os
from contextlib import ExitStack

import numpy as np

import concourse.bass as bass
import concourse.bacc as bacc
import concourse.tile as tile
from concourse import mybir
from concourse.bass_utils import run_bass_kernel_spmd
from concourse.masks import make_identity

N_CORES = 8
B, S, D = 4, 4096, 1024
F = 1024
ROWS_TOTAL = B * S          # 16384
ROWS = ROWS_TOTAL // N_CORES  # 2048 per core
P = 128
NSPLIT = 512                # one PSUM bank of f32


def build_nc(rows: int = ROWS) -> bass.Bass:
    nc = bacc.Bacc("TRN2", target_bir_lowering=False, debug=False)
    x = nc.dram_tensor("x", [rows, D], mybir.dt.float32, kind="ExternalInput").ap()
    W = nc.dram_tensor("W", [D, F], mybir.dt.float32, kind="ExternalInput").ap()
    b = nc.dram_tensor("b", [F], mybir.dt.float32, kind="ExternalInput").ap()
    y = nc.dram_tensor("y", [rows, F], mybir.dt.float32, kind="ExternalOutput").ap()

    KB = D // P        # 8 contraction blocks
    NB = F // NSPLIT   # 2 psum banks per rowblock
    RB = rows // P     # rowblocks

    with tile.TileContext(nc) as tc, ExitStack() as ctx:
        const = ctx.enter_context(tc.tile_pool(name="const", bufs=1))
        xp = ctx.enter_context(tc.tile_pool(name="xp", bufs=RB))
        xtp = ctx.enter_context(tc.tile_pool(name="xtp", bufs=RB))
        yp = ctx.enter_context(tc.tile_pool(name="yp", bufs=RB))
        psp = ctx.enter_context(tc.tile_pool(name="psp", bufs=1, space="PSUM"))

        # W, cast to bf16, laid out [p, k, f] with d = k*128 + p to match
        # the xbar-transpose output layout of x.  Split per k-block so the
        # first matmuls' weights land early.
        W_bf = const.tile([P, KB, F], mybir.dt.bfloat16)
        W_pkf = W.rearrange("(k p) f -> p k f", p=P)

        # Bias broadcast to all partitions, f32.
        b_bc = const.tile([P, F], mybir.dt.float32)

        # Identity for PE-based transposes.
        ident = const.tile([P, P], mybir.dt.bfloat16)
        make_identity(nc, ident[:])

        # HAM warmup: ~10 cold matmuls (~4.3us of PE activity) on a zeroed
        # tile flip the PE clock gate to 8/8 while the first DMAs land.
        warm = const.tile([P, P], mybir.dt.bfloat16)
        nc.any.memset(warm[:], 0.0)
        warm_ps = psp.tile([P, NSPLIT], mybir.dt.float32, tag="ps0", bufs=2)
        for _ in range(10):
            nc.tensor.matmul(warm_ps[:], warm[:], warm[:, 0:1].to_broadcast([P, NSPLIT]),
                             start=True, stop=True, skip_group_check=True)

        x_tiles = []
        nc.gpsimd.dma_start(W_bf[:, 0, :], W_pkf[:, 0, :])
        for rb in range(2):
            x_bf = xp.tile([P, D], mybir.dt.bfloat16, name="x_bf", tag="x_bf")
            nc.gpsimd.dma_start(x_bf[:], x[rb * P:(rb + 1) * P, :])  # cast load
            x_tiles.append(x_bf)
        nc.gpsimd.dma_start(b_bc[:], b.rearrange("(o f) -> o f", o=1).to_broadcast([P, F]))
        for k in range(1, KB):
            nc.gpsimd.dma_start(W_bf[:, k, :], W_pkf[:, k, :])
        for rb in range(2, RB):
            x_bf = xp.tile([P, D], mybir.dt.bfloat16, name="x_bf", tag="x_bf")
            nc.gpsimd.dma_start(x_bf[:], x[rb * P:(rb + 1) * P, :])
            x_tiles.append(x_bf)

        for rb in range(RB):
            x_bf = x_tiles[rb]
            # Transpose the 8 k-tiles on the PE into one PSUM bank, then one
            # copyback into SBUF.  xT[p, k, j] = x_bf[j, k*128+p].
            psT = psp.tile([P, KB, P], mybir.dt.bfloat16, name="psT", tag="psT", bufs=2)
            for k in range(KB):
                nc.tensor.transpose(psT[:, k, :], x_bf[:, k * P:(k + 1) * P], ident[:])
            xT = xtp.tile([P, KB, P], mybir.dt.bfloat16, name="xT", tag="xT")
            if rb % 2 == 0:
                nc.scalar.copy(xT[:], psT[:])
            else:
                nc.vector.tensor_copy(xT[:], psT[:])

            y_sb = yp.tile([P, F], mybir.dt.float32)
            pss = [psp.tile([P, NSPLIT], mybir.dt.float32, name=f"ps{n}", tag=f"ps{n}", bufs=2) for n in range(NB)]
            for k in range(KB):
                for n in range(NB):
                    nc.tensor.matmul(
                        pss[n][:],
                        xT[:, k, :],
                        W_bf[:, k, n * NSPLIT:(n + 1) * NSPLIT],
                        start=(k == 0),
                        stop=(k == KB - 1),
                    )
            for n in range(NB):
                nc.vector.tensor_add(
                    y_sb[:, n * NSPLIT:(n + 1) * NSPLIT],
                    pss[n][:],
                    b_bc[:, n * NSPLIT:(n + 1) * NSPLIT],
                )
            nc.scalar.dma_start(y[rb * P:(rb + 1) * P, :], y_sb[:])

    nc.compile()
    return nc


_NC_CACHE: dict[int, bass.Bass] = {}


def _get_nc(rows: int = ROWS) -> bass.Bass:
    if rows not in _NC_CACHE:
        _NC_CACHE[rows] = build_nc(rows)
    return _NC_CACHE[rows]


def _run(in_maps, rows: int = ROWS, trace: bool = False):
    nc = _get_nc(rows)
    return run_bass_kernel_spmd(nc, in_maps, list(range(N_CORES)), trace=trace)


def kernel(x: np.ndarray, W: np.ndarray, b: np.ndarray) -> np.ndarray:
    x = np.ascontiguousarray(np.asarray(x, dtype=np.float32))
    W = np.ascontiguousarray(np.asarray(W, dtype=np.float32))
    b = np.ascontiguousarray(np.asarray(b, dtype=np.float32))
    x_flat = x.reshape(ROWS_TOTAL, D)
    in_maps = [
        {"x": np.ascontiguousarray(x_flat[c * ROWS:(c + 1) * ROWS]), "W": W, "b": b}
        for c in range(N_CORES)
    ]
    res = _run(in_maps, trace=bool(int(os.environ.get("BASS_KERNEL_TRACE", "0"))))
    y = np.concatenate([res.results[c]["y"] for c in range(N_CORES)], axis=0)
    return y.reshape(B, S, F)
